# revision 21
# baseline (speedup 1.0000x reference)
"""Multi-head self-attention on 8 Trainium2 NeuronCores (raw Bass).

Problem: B=2, S=2048, D=1024, H=16 heads of depth 64 (fp32).
    q/k/v = x @ W.T + b ; per-head softmax(q k^T / 8) v ; dense out proj.

Sharding: DP-2 on batch x TP-4 on heads. Core (b, g) handles batch b and the
4 heads g*4..g*4+3 (a 256-wide column block of q/k/v). The dense layer is
row-split over the same block, so each core emits a partial [S, D] output;
the host sums the 4 partials per batch (dense bias rides on the g==0 cores).

Raw-bass implementation (the toolchain accepts at most ONE semaphore wait
per engine instruction, so Tile's multi-wait sync is unusable; explicit
wait_ge instructions + then_inc updates are used throughout):

  phase 1: stream x^T in [128, 512] slices; QT/KT/VT = W_T.T @ x^T in [c, s]
           layout (f32r matmuls, 6 psum accumulators); VT slices are
           PE-transposed into V_aug [s, c] with a ones column appended.
  attention (8 iters = head pair x 512-wide q block; 16 k tiles each):
           scores^T via row-packed pair matmuls (contraction=64, two heads
           share the 128-row PE array), exp on ScalarE (scale=1/8 folded in),
           PV accumulation with lhsT = V_aug; its ones column leaves the
           softmax denominators in psum row 64; normalize via reciprocal +
           a DRAM-bounce partition broadcast.
  dense:   per q block, partial^T = dwT.T @ OT (+ db) -> DMA out.
"""

import numpy as np
import sys

if "/opt/trn_rl_repo" not in sys.path:
    sys.path.insert(0, "/opt/trn_rl_repo")

import concourse.bass as bass
from concourse import mybir
from concourse.bass_utils import run_bass_kernel_spmd

F32 = mybir.dt.float32
F32R = mybir.dt.float32r
AFT = mybir.ActivationFunctionType

B, S, D = 2, 2048, 1024
H, DEPTH = 16, 64
TP = 4                     # head-parallel groups
C = D // TP                # 256 cols per core (4 heads)
CT = C // 128              # 2 partition tiles of the head block
KD = D // 128              # 8 contraction tiles for projections
ST_N = S // 128            # 16 s/k tiles
QB = 512                   # q block width
NQB = S // QB              # 4 q blocks
NPAIR = 2                  # head pairs per core
NITER = NQB * NPAIR        # 8 attention iterations (qp)
NE = NITER * ST_N          # 128 score-tile steps (e)
NX = 4                     # x slice ring depth
NP = 4                     # p tile ring depth
SCALE = float(1.0 / np.sqrt(DEPTH))


def build_nc():
    nc = bass.Bass()

    xT = nc.dram_tensor("xT", [D, S], F32R, kind="ExternalInput")
    wqT = nc.dram_tensor("wqT", [D, C], F32R, kind="ExternalInput")
    wkT = nc.dram_tensor("wkT", [D, C], F32R, kind="ExternalInput")
    wvT = nc.dram_tensor("wvT", [D, C], F32R, kind="ExternalInput")
    bq = nc.dram_tensor("bq", [C, 1], F32, kind="ExternalInput")
    bk = nc.dram_tensor("bk", [C, 1], F32, kind="ExternalInput")
    bv = nc.dram_tensor("bv", [C, 1], F32, kind="ExternalInput")
    dwT = nc.dram_tensor("dwT", [C, D], F32R, kind="ExternalInput")
    db = nc.dram_tensor("db", [D, 1], F32, kind="ExternalInput")
    identity = nc.dram_tensor("identity", [128, 128], F32R, kind="ExternalInput")
    outT = nc.dram_tensor("outT", [D, S], F32, kind="ExternalOutput")
    inv_scr = nc.dram_tensor("inv_scr", [NITER, 2, 1, QB], F32)

    # ---- SBUF ----
    wq_sb = nc.alloc_sbuf_tensor("wq_sb", [128, KD, C], F32R).ap()
    wk_sb = nc.alloc_sbuf_tensor("wk_sb", [128, KD, C], F32R).ap()
    wv_sb = nc.alloc_sbuf_tensor("wv_sb", [128, KD, C], F32R).ap()
    bq_sb = nc.alloc_sbuf_tensor("bq_sb", [128, CT, 1], F32).ap()
    bk_sb = nc.alloc_sbuf_tensor("bk_sb", [128, CT, 1], F32).ap()
    bv_sb = nc.alloc_sbuf_tensor("bv_sb", [128, CT, 1], F32).ap()
    dw_sb = nc.alloc_sbuf_tensor("dw_sb", [128, CT, D], F32R).ap()
    db_sb = nc.alloc_sbuf_tensor("db_sb", [128, KD, 1], F32).ap()
    ident = nc.alloc_sbuf_tensor("ident", [128, 128], F32R).ap()
    x_ring = nc.alloc_sbuf_tensor("x_ring", [128, NX, QB], F32R).ap()
    qt_sb = nc.alloc_sbuf_tensor("qt_sb", [128, CT, S], F32R).ap()
    kt_sb = nc.alloc_sbuf_tensor("kt_sb", [128, CT, S], F32R).ap()
    vt_roll = nc.alloc_sbuf_tensor("vt_roll", [128, 2, QB], F32R).ap()
    vaug = nc.alloc_sbuf_tensor("vaug", [128, ST_N, TP, DEPTH + 1], F32R).ap()
    p_ring = nc.alloc_sbuf_tensor("p_ring", [128, NP, 2, QB], F32R).ap()
    inv_sb = nc.alloc_sbuf_tensor("inv_sb", [128, 2, QB], F32).ap()
    invb = nc.alloc_sbuf_tensor("invb", [64, 2, QB], F32).ap()
    tmp_sb = nc.alloc_sbuf_tensor("tmp_sb", [64, 2, QB], F32R).ap()
    ot = nc.alloc_sbuf_tensor("ot", [128, 2, CT, QB], F32R).ap()
    stage = nc.alloc_sbuf_tensor("stage", [128, 2, KD, QB], F32).ap()

    # ---- PSUM: one [128, 8, 512] tensor, banks managed manually ----
    # phase 1: banks 0-5 = projection accumulators (w*2+ct), 6-7 = transposes
    # attention: banks 0-3 = score tiles (slot b*2+head), 4-5 = PV accum A/B,
    #            6-7 = dense ring
    psum = nc.alloc_psum_tensor("ps", [128, 8, QB], F32).ap()

    # ---- semaphores ----
    # DMA completions across HW queues are out-of-order, so every DMA
    # stream with >1 outstanding transfer gets per-slot semaphores: each
    # wait value then corresponds to a deterministic set of completions.
    s = {n: nc.alloc_semaphore(n) for n in (
        "s_in", "s_xcons", "s_cp", "s_tr", "s_trcp",
        "s_st", "s_exp", "s_pcons", "s_nrm", "s_inv", "s_dn", "s_stg")}
    s_x = [nc.alloc_semaphore(f"s_x{j}") for j in range(NX)]
    s_invs = [nc.alloc_semaphore(f"s_invs{h}") for h in range(2)]
    s_invb = [nc.alloc_semaphore(f"s_invb{h}") for h in range(2)]
    s_ot = [nc.alloc_semaphore(f"s_ot{p}") for p in range(2)]
    s_out = [nc.alloc_semaphore(f"s_out{p}") for p in range(2)]

    projs = ((wq_sb, bq_sb, 0), (wk_sb, bk_sb, 1), (wv_sb, bv_sb, 2))

    with nc.Block() as block:

        # ---------------- SP: all HWDGE DMA traffic ----------------
        @block.sync
        def _(sync):
            # 9 input DMAs -> s_in = 144
            sync.dma_start(
                out=wq_sb, in_=wqT.ap().rearrange("(k p) c -> p k c", p=128)
            ).then_inc(s["s_in"], 16)
            sync.dma_start(
                out=wk_sb, in_=wkT.ap().rearrange("(k p) c -> p k c", p=128)
            ).then_inc(s["s_in"], 16)
            sync.dma_start(
                out=wv_sb, in_=wvT.ap().rearrange("(k p) c -> p k c", p=128)
            ).then_inc(s["s_in"], 16)
            with nc.allow_non_contiguous_dma(reason="tiny bias vectors"):
                sync.dma_start(
                    out=bq_sb, in_=bq.ap().rearrange("(ct p) o -> p ct o", p=128)
                ).then_inc(s["s_in"], 16)
                sync.dma_start(
                    out=bk_sb, in_=bk.ap().rearrange("(ct p) o -> p ct o", p=128)
                ).then_inc(s["s_in"], 16)
                sync.dma_start(
                    out=bv_sb, in_=bv.ap().rearrange("(ct p) o -> p ct o", p=128)
                ).then_inc(s["s_in"], 16)
            sync.dma_start(
                out=dw_sb, in_=dwT.ap().rearrange("(ct p) e -> p ct e", p=128)
            ).then_inc(s["s_in"], 16)
            with nc.allow_non_contiguous_dma(reason="tiny bias vector"):
                sync.dma_start(
                    out=db_sb, in_=db.ap().rearrange("(m p) o -> p m o", p=128)
                ).then_inc(s["s_in"], 16)
            sync.dma_start(out=ident, in_=identity.ap()).then_inc(s["s_in"], 16)

            # x slices: i = n*KD + k
            for n in range(NQB):
                for k in range(KD):
                    i = n * KD + k
                    if i >= NX:
                        sync.wait_ge(s["s_xcons"], i - (NX - 1))
                    sync.dma_start(
                        out=x_ring[:, i % NX, :],
                        in_=xT[k * 128:(k + 1) * 128, n * QB:(n + 1) * QB],
                    ).then_inc(s_x[i % NX], 16)

            # attention-side DMA chains
            for qb in range(NQB):
                for pair in range(NPAIR):
                    qp = qb * NPAIR + pair
                    for h in range(2):
                        sync.wait_ge(s["s_inv"], 2 * qp + h + 1)
                        sync.dma_start(
                            out=inv_scr.ap()[qp, h, :, :],
                            in_=inv_sb[64:65, h, :],
                        ).then_inc(s_invs[h], 16)
                    for h in range(2):
                        if qp >= 1:
                            # WAR: previous mul must have read invb[h]
                            sync.wait_ge(s["s_nrm"], 2 * (qp - 1) + h + 1)
                        sync.wait_ge(s_invs[h], 16 * (qp + 1))
                        sync.dma_start(
                            out=invb[:, h, :],
                            in_=inv_scr.ap()[qp, h, :, :].to_broadcast([64, QB]),
                        ).then_inc(s_invb[h], 16)
                    # head B partition shift tmp -> ot rows 64..127
                    if qb >= 2 and pair == 0:
                        sync.wait_ge(s["s_dn"], 8 * (qb - 1))
                    sync.wait_ge(s["s_nrm"], 2 * qp + 2)
                    sync.dma_start(
                        out=ot[64:128, qb % 2, pair, :], in_=tmp_sb[:, qp % 2, :]
                    ).then_inc(s_ot[qp % 2], 16)
                # output DMAs
                for m8 in range(KD):
                    d = qb * KD + m8
                    sync.wait_ge(s["s_stg"], d + 1)
                    sync.dma_start(
                        out=outT[m8 * 128:(m8 + 1) * 128, qb * QB:(qb + 1) * QB],
                        in_=stage[:, qb % 2, m8, :],
                    ).then_inc(s_out[qb % 2], 16)

        # ---------------- PE: matmuls + transposes ----------------
        @block.tensor
        def _(tensor):
            tensor.wait_ge(s["s_in"], 144)
            # phase 1
            for n in range(NQB):
                nsl = slice(n * QB, (n + 1) * QB)
                if n >= 1:
                    tensor.wait_ge(s["s_cp"], 6 * n)  # psum accumulators freed
                for k in range(KD):
                    i = n * KD + k
                    tensor.wait_ge(s_x[i % NX], 16 * (i // NX + 1))
                    last = None
                    for w_sb, _, w in projs:
                        for ct in range(CT):
                            last = nc.tensor.matmul(
                                psum[:, w * 2 + ct, :],
                                w_sb[:, k, ct * 128:(ct + 1) * 128],
                                x_ring[:, i % NX, :],
                                start=(k == 0), stop=(k == KD - 1),
                            )
                    last.then_inc(s["s_xcons"], 1)
                # V transposes for this n: t = n*8 + ct*4 + j
                for ct in range(CT):
                    tensor.wait_ge(s["s_cp"], 6 * n + 5 + ct)  # vt_roll ready
                    for j in range(QB // 128):
                        t = n * (2 * (QB // 128)) + ct * (QB // 128) + j
                        if t >= 2:
                            tensor.wait_ge(s["s_trcp"], t - 1)
                        nc.tensor.transpose(
                            psum[:, 6 + t % 2, 0:128].bitcast(F32R),
                            vt_roll[:, (2 * n + ct) % 2, j * 128:(j + 1) * 128],
                            ident,
                        ).then_inc(s["s_tr"], 1)

            # phase 1 psum fully consumed before attention reuses the banks
            tensor.wait_ge(s["s_cp"], 6 * NQB)
            tensor.wait_ge(s["s_trcp"], 2 * (QB // 128) * NQB)

            # attention
            for qb in range(NQB):
                qsl = slice(qb * QB, (qb + 1) * QB)
                for pair in range(NPAIR):
                    qp = qb * NPAIR + pair
                    for m in range(ST_N):
                        e = qp * ST_N + m
                        b = e % 2
                        msl = slice(m * 128, (m + 1) * 128)
                        # scores^T, two heads row-packed
                        if e >= 2:
                            tensor.wait_ge(s["s_exp"], e - 1)
                        nc.tensor.matmul(
                            psum[:, b * 2 + 0, :],
                            kt_sb[0:64, pair, msl],
                            qt_sb[0:64, pair, qsl],
                            start=True, stop=True, tile_position=(0, 0),
                        )
                        nc.tensor.matmul(
                            psum[:, b * 2 + 1, :],
                            kt_sb[64:128, pair, msl],
                            qt_sb[64:128, pair, qsl],
                            start=True, stop=True, tile_position=(64, 0),
                        ).then_inc(s["s_st"], 1)
                        # PV accumulation (ones column -> denominators)
                        tensor.wait_ge(s["s_exp"], e + 1)
                        if m == 0 and qp >= 1:
                            tensor.wait_ge(s["s_nrm"], 2 * qp)
                        nc.tensor.matmul(
                            psum[0:65, 4, :],
                            vaug[:, m, 2 * pair, :],
                            p_ring[:, e % NP, 0, :],
                            start=(m == 0), stop=(m == ST_N - 1),
                        )
                        nc.tensor.matmul(
                            psum[0:65, 5, :],
                            vaug[:, m, 2 * pair + 1, :],
                            p_ring[:, e % NP, 1, :],
                            start=(m == 0), stop=(m == ST_N - 1),
                        ).then_inc(s["s_pcons"], 1)
                # dense for this q block
                tensor.wait_ge(s["s_nrm"], 4 * qb + 4)
                tensor.wait_ge(s_ot[0], 16 * (qb + 1))
                tensor.wait_ge(s_ot[1], 16 * (qb + 1))
                for m8 in range(KD):
                    d = qb * KD + m8
                    if d >= 2:
                        tensor.wait_ge(s["s_stg"], d - 1)
                    nc.tensor.matmul(
                        psum[:, 6 + d % 2, :],
                        dw_sb[:, 0, m8 * 128:(m8 + 1) * 128],
                        ot[:, qb % 2, 0, :],
                        start=True, stop=False,
                    )
                    nc.tensor.matmul(
                        psum[:, 6 + d % 2, :],
                        dw_sb[:, 1, m8 * 128:(m8 + 1) * 128],
                        ot[:, qb % 2, 1, :],
                        start=False, stop=True,
                    ).then_inc(s["s_dn"], 1)

        # ---------------- ACT: exp ----------------
        @block.scalar
        def _(scalar):
            for e in range(NE):
                b = e % 2
                scalar.wait_ge(s["s_st"], e + 1)
                if e >= NP:
                    scalar.wait_ge(s["s_pcons"], e - (NP - 1))
                nc.scalar.activation(
                    out=p_ring[:, e % NP, :, :],
                    in_=psum[:, b * 2:b * 2 + 2, :],
                    func=AFT.Exp, scale=SCALE,
                ).then_inc(s["s_exp"], 1)

        # ---------------- DVE: bias adds, copies, normalize, stage ----------
        @block.vector
        def _(vector):
            for st_i in range(ST_N):
                for hh in range(TP):
                    nc.vector.memset(
                        vaug[:, st_i, hh, DEPTH:DEPTH + 1].bitcast(F32), 1.0
                    )
            vector.wait_ge(s["s_in"], 144)
            # phase 1
            for n in range(NQB):
                nsl = slice(n * QB, (n + 1) * QB)
                vector.wait_ge(s["s_xcons"], KD * (n + 1))
                for w_sb, b_sb, w in projs[:2]:
                    dst = qt_sb if w == 0 else kt_sb
                    for ct in range(CT):
                        nc.vector.tensor_scalar_add(
                            out=dst[:, ct, nsl],
                            in0=psum[:, w * 2 + ct, :],
                            scalar1=b_sb[:, ct, :],
                        ).then_inc(s["s_cp"], 1)
                for ct in range(CT):
                    # vt_roll WAR: transposes of the slot two groups back
                    g = 2 * n + ct
                    if g >= 2:
                        prev = g - 2  # = 2n'+ct'
                        tprev = (prev // 2) * (2 * (QB // 128)) \
                            + (prev % 2) * (QB // 128) + (QB // 128 - 1)
                        vector.wait_ge(s["s_tr"], tprev + 1)
                    nc.vector.tensor_scalar_add(
                        out=vt_roll[:, g % 2, :],
                        in0=psum[:, 4 + ct, :],
                        scalar1=bv_sb[:, ct, :],
                    ).then_inc(s["s_cp"], 1)
                # V_aug assembly from transposed tiles
                for ct in range(CT):
                    for j in range(QB // 128):
                        t = n * (2 * (QB // 128)) + ct * (QB // 128) + j
                        st_i = n * (QB // 128) + j
                        vector.wait_ge(s["s_tr"], t + 1)
                        nc.vector.tensor_copy(
                            out=vaug[:, st_i, 2 * ct, 0:DEPTH],
                            in_=psum[:, 6 + t % 2, 0:DEPTH],
                        )
                        nc.vector.tensor_copy(
                            out=vaug[:, st_i, 2 * ct + 1, 0:DEPTH],
                            in_=psum[:, 6 + t % 2, DEPTH:128],
                        ).then_inc(s["s_trcp"], 1)

            # attention: normalization + dense staging
            for qb in range(NQB):
                for pair in range(NPAIR):
                    qp = qb * NPAIR + pair
                    vector.wait_ge(s["s_pcons"], ST_N * (qp + 1))
                    for h in range(2):
                        if qp >= 1:
                            # inv_sb WAR: scratch DMA of previous qp done
                            vector.wait_ge(s_invs[h], 16 * qp)
                        nc.vector.reciprocal(
                            out=inv_sb[64:65, h, :], in_=psum[64:65, 4 + h, :]
                        ).then_inc(s["s_inv"], 1)
                    for h in range(2):
                        if pair == 0 and h == 0 and qb >= 2:
                            vector.wait_ge(s["s_dn"], 8 * (qb - 1))  # ot WAR
                        vector.wait_ge(s_invb[h], 16 * (qp + 1))
                        if h == 0:
                            nc.vector.tensor_mul(
                                out=ot[0:64, qb % 2, pair, :],
                                in0=psum[0:64, 4, :], in1=invb[:, 0, :],
                            ).then_inc(s["s_nrm"], 1)
                        else:
                            if qp >= 2:
                                vector.wait_ge(s_ot[qp % 2], 16 * (qp // 2))
                            nc.vector.tensor_mul(
                                out=tmp_sb[:, qp % 2, :],
                                in0=psum[0:64, 5, :], in1=invb[:, 1, :],
                            ).then_inc(s["s_nrm"], 1)
                for m8 in range(KD):
                    d = qb * KD + m8
                    vector.wait_ge(s["s_dn"], d + 1)
                    if qb >= 2 and m8 == 0:
                        # stage slot WAR: all of q block qb-2's output DMAs
                        vector.wait_ge(s_out[qb % 2], 16 * KD * (qb // 2))
                    nc.vector.tensor_scalar_add(
                        out=stage[:, qb % 2, m8, :],
                        in0=psum[:, 6 + d % 2, :],
                        scalar1=db_sb[:, m8, :],
                    ).then_inc(s["s_stg"], 1)

    nc.finalize()
    return nc


_NC_CACHE = []


def get_nc():
    if not _NC_CACHE:
        _NC_CACHE.append(build_nc())
    return _NC_CACHE[0]


def make_in_maps(x, wq_w, wq_b, wk_w, wk_b, wv_w, wv_b, dense_w, dense_b):
    in_maps = []
    for core in range(8):
        b, g = divmod(core, TP)
        blk = slice(g * C, (g + 1) * C)
        db_g = dense_b if g == 0 else np.zeros_like(dense_b)
        in_maps.append({
            "xT": np.ascontiguousarray(x[b].T),
            "wqT": np.ascontiguousarray(wq_w[blk, :].T),
            "wkT": np.ascontiguousarray(wk_w[blk, :].T),
            "wvT": np.ascontiguousarray(wv_w[blk, :].T),
            "bq": np.ascontiguousarray(wq_b[blk].reshape(C, 1)),
            "bk": np.ascontiguousarray(wk_b[blk].reshape(C, 1)),
            "bv": np.ascontiguousarray(wv_b[blk].reshape(C, 1)),
            "dwT": np.ascontiguousarray(dense_w[:, blk].T),
            "db": np.ascontiguousarray(db_g.reshape(D, 1)),
            "identity": np.eye(128, dtype=np.float32),
        })
    return in_maps


def gather_out(results):
    out = np.zeros((B, S, D), dtype=np.float32)
    for core in range(8):
        b = core // TP
        out[b] += results[core]["outT"].T
    return out


def kernel(x, wq_w, wq_b, wk_w, wk_b, wv_w, wv_b, dense_w, dense_b, **run_kwargs):
    args = [np.asarray(a, dtype=np.float32) for a in (
        x, wq_w, wq_b, wk_w, wk_b, wv_w, wv_b, dense_w, dense_b)]
    nc = get_nc()
    in_maps = make_in_maps(*args)
    res = run_bass_kernel_spmd(nc, in_maps, list(range(8)), **run_kwargs)
    out = gather_out(res.results)
    kernel.last_results = res
    return out


# revision 23
# speedup vs baseline: 1.7555x; 1.7555x over previous
"""Multi-head self-attention on 8 Trainium2 NeuronCores (raw Bass).

Problem: B=2, S=2048, D=1024, H=16 heads of depth 64 (fp32).
    q/k/v = x @ W.T + b ; per-head softmax(q k^T / 8) v ; dense out proj.

Sharding: DP-2 on batch x TP-4 on heads. Core (b, g) handles batch b and the
4 heads g*4..g*4+3 (a 256-wide column block of q/k/v). The dense layer is
row-split over the same block, so each core emits a partial [S, D] output;
the host sums the 4 partials per batch (dense bias rides on the g==0 cores).

Raw-bass implementation (the toolchain accepts at most ONE semaphore wait
per engine instruction, so Tile's multi-wait sync is unusable; explicit
wait_ge instructions + then_inc updates are used throughout):

  phase 1: stream x^T in [128, 512] slices; QT/KT/VT = W_T.T @ x^T in [c, s]
           layout (f32r matmuls, 6 psum accumulators); VT slices are
           PE-transposed into V_aug [s, c] with a ones column appended.
  attention (8 iters = head pair x 512-wide q block; 16 k tiles each):
           scores^T via row-packed pair matmuls (contraction=64, two heads
           share the 128-row PE array), exp on ScalarE (scale=1/8 folded in),
           PV accumulation with lhsT = V_aug; its ones column leaves the
           softmax denominators in psum row 64; normalize via reciprocal +
           a DRAM-bounce partition broadcast.
  dense:   per q block, partial^T = dwT.T @ OT (+ db) -> DMA out.
"""

import numpy as np
import sys

if "/opt/trn_rl_repo" not in sys.path:
    sys.path.insert(0, "/opt/trn_rl_repo")

import concourse.bass as bass
from concourse import mybir
from concourse.bass_utils import run_bass_kernel_spmd

F32 = mybir.dt.float32
F32R = mybir.dt.float32r
AFT = mybir.ActivationFunctionType

B, S, D = 2, 2048, 1024
H, DEPTH = 16, 64
TP = 4                     # head-parallel groups
C = D // TP                # 256 cols per core (4 heads)
CT = C // 128              # 2 partition tiles of the head block
KD = D // 128              # 8 contraction tiles for projections
ST_N = S // 128            # 16 s/k tiles
QB = 512                   # q block width
NQB = S // QB              # 4 q blocks
NPAIR = 2                  # head pairs per core
NITER = NQB * NPAIR        # 8 attention iterations (qp)
NE = NITER * ST_N          # 128 score-tile steps (e)
NX = 4                     # x slice ring depth
NP = 4                     # p tile ring depth
SCALE = float(1.0 / np.sqrt(DEPTH))


def build_nc():
    nc = bass.Bass()

    xT = nc.dram_tensor("xT", [D, S], F32R, kind="ExternalInput")
    wqT = nc.dram_tensor("wqT", [D, C], F32R, kind="ExternalInput")
    wkT = nc.dram_tensor("wkT", [D, C], F32R, kind="ExternalInput")
    wvT = nc.dram_tensor("wvT", [D, C], F32R, kind="ExternalInput")
    bq = nc.dram_tensor("bq", [C, 1], F32, kind="ExternalInput")
    bk = nc.dram_tensor("bk", [C, 1], F32, kind="ExternalInput")
    bv = nc.dram_tensor("bv", [C, 1], F32, kind="ExternalInput")
    dwT = nc.dram_tensor("dwT", [C, D], F32R, kind="ExternalInput")
    db = nc.dram_tensor("db", [D, 1], F32, kind="ExternalInput")
    identity = nc.dram_tensor("identity", [128, 128], F32R, kind="ExternalInput")
    outT = nc.dram_tensor("outT", [D, S], F32, kind="ExternalOutput")
    inv_scr = nc.dram_tensor("inv_scr", [NITER, 2, 1, QB], F32)

    # ---- SBUF ----
    wq_sb = nc.alloc_sbuf_tensor("wq_sb", [128, KD, C], F32R).ap()
    wk_sb = nc.alloc_sbuf_tensor("wk_sb", [128, KD, C], F32R).ap()
    wv_sb = nc.alloc_sbuf_tensor("wv_sb", [128, KD, C], F32R).ap()
    bq_sb = nc.alloc_sbuf_tensor("bq_sb", [128, CT, 1], F32).ap()
    bk_sb = nc.alloc_sbuf_tensor("bk_sb", [128, CT, 1], F32).ap()
    bv_sb = nc.alloc_sbuf_tensor("bv_sb", [128, CT, 1], F32).ap()
    dw_sb = nc.alloc_sbuf_tensor("dw_sb", [128, CT, D], F32R).ap()
    db_sb = nc.alloc_sbuf_tensor("db_sb", [128, KD, 1], F32).ap()
    ident = nc.alloc_sbuf_tensor("ident", [128, 128], F32R).ap()
    x_ring = nc.alloc_sbuf_tensor("x_ring", [128, NX, QB], F32R).ap()
    qt_sb = nc.alloc_sbuf_tensor("qt_sb", [128, CT, S], F32R).ap()
    kt_sb = nc.alloc_sbuf_tensor("kt_sb", [128, CT, S], F32R).ap()
    vt_roll = nc.alloc_sbuf_tensor("vt_roll", [128, 2, QB], F32R).ap()
    vaug = nc.alloc_sbuf_tensor("vaug", [128, ST_N, TP, DEPTH + 1], F32R).ap()
    p_ring = nc.alloc_sbuf_tensor("p_ring", [128, NP, 2, QB], F32R).ap()
    inv_sb = nc.alloc_sbuf_tensor("inv_sb", [128, 2, QB], F32).ap()
    invb = nc.alloc_sbuf_tensor("invb", [64, 2, QB], F32).ap()
    tmp_sb = nc.alloc_sbuf_tensor("tmp_sb", [64, 2, QB], F32R).ap()
    ot = nc.alloc_sbuf_tensor("ot", [128, 2, CT, QB], F32R).ap()
    ounorm = nc.alloc_sbuf_tensor("ounorm", [128, 2, 2, QB], F32).ap()
    stage = nc.alloc_sbuf_tensor("stage", [128, 2, KD, QB], F32).ap()

    # ---- PSUM: one [128, 8, 512] tensor, banks managed manually ----
    # phase 1: banks 0-5 = projection accumulators (w*2+ct), 6-7 = transposes
    # attention: banks 0-3 = score tiles (slot b*2+head), 4-5 = PV accum A/B,
    #            6-7 = dense ring
    psum = nc.alloc_psum_tensor("ps", [128, 8, QB], F32).ap()

    # ---- semaphores ----
    # DMA completions across HW queues are out-of-order, so every DMA
    # stream with >1 outstanding transfer gets per-slot semaphores: each
    # wait value then corresponds to a deterministic set of completions.
    s = {n: nc.alloc_semaphore(n) for n in (
        "s_in", "s_xcons", "s_cp", "s_tr", "s_trcp",
        "s_st", "s_exp", "s_pcons", "s_acs", "s_nrm", "s_inv", "s_dn",
        "s_stg")}
    s_x = [nc.alloc_semaphore(f"s_x{j}") for j in range(NX)]
    s_invs = [nc.alloc_semaphore(f"s_invs{h}") for h in range(2)]
    s_invb = [nc.alloc_semaphore(f"s_invb{h}") for h in range(2)]
    s_ot = [nc.alloc_semaphore(f"s_ot{p}") for p in range(2)]
    s_out = [nc.alloc_semaphore(f"s_out{p}") for p in range(2)]

    projs = ((wq_sb, bq_sb, 0), (wk_sb, bk_sb, 1), (wv_sb, bv_sb, 2))

    with nc.Block() as block:

        # ---------------- SP: all HWDGE DMA traffic ----------------
        @block.sync
        def _(sync):
            # 9 input DMAs -> s_in = 144
            sync.dma_start(
                out=wq_sb, in_=wqT.ap().rearrange("(k p) c -> p k c", p=128)
            ).then_inc(s["s_in"], 16)
            sync.dma_start(
                out=wk_sb, in_=wkT.ap().rearrange("(k p) c -> p k c", p=128)
            ).then_inc(s["s_in"], 16)
            sync.dma_start(
                out=wv_sb, in_=wvT.ap().rearrange("(k p) c -> p k c", p=128)
            ).then_inc(s["s_in"], 16)
            with nc.allow_non_contiguous_dma(reason="tiny bias vectors"):
                sync.dma_start(
                    out=bq_sb, in_=bq.ap().rearrange("(ct p) o -> p ct o", p=128)
                ).then_inc(s["s_in"], 16)
                sync.dma_start(
                    out=bk_sb, in_=bk.ap().rearrange("(ct p) o -> p ct o", p=128)
                ).then_inc(s["s_in"], 16)
                sync.dma_start(
                    out=bv_sb, in_=bv.ap().rearrange("(ct p) o -> p ct o", p=128)
                ).then_inc(s["s_in"], 16)
            sync.dma_start(
                out=dw_sb, in_=dwT.ap().rearrange("(ct p) e -> p ct e", p=128)
            ).then_inc(s["s_in"], 16)
            with nc.allow_non_contiguous_dma(reason="tiny bias vector"):
                sync.dma_start(
                    out=db_sb, in_=db.ap().rearrange("(m p) o -> p m o", p=128)
                ).then_inc(s["s_in"], 16)
            sync.dma_start(out=ident, in_=identity.ap()).then_inc(s["s_in"], 16)

            # x slices: i = n*KD + k
            for n in range(NQB):
                for k in range(KD):
                    i = n * KD + k
                    if i >= NX:
                        sync.wait_ge(s["s_xcons"], i - (NX - 1))
                    sync.dma_start(
                        out=x_ring[:, i % NX, :],
                        in_=xT[k * 128:(k + 1) * 128, n * QB:(n + 1) * QB],
                    ).then_inc(s_x[i % NX], 16)

            # attention-side DMA chains
            for qb in range(NQB):
                for pair in range(NPAIR):
                    qp = qb * NPAIR + pair
                    for h in range(2):
                        sync.wait_ge(s["s_inv"], 2 * qp + h + 1)
                        sync.dma_start(
                            out=inv_scr.ap()[qp, h, :, :],
                            in_=inv_sb[64:65, h, :],
                        ).then_inc(s_invs[h], 16)
                    for h in range(2):
                        if qp >= 1:
                            # WAR: previous mul must have read invb[h]
                            sync.wait_ge(s["s_nrm"], 2 * (qp - 1) + h + 1)
                        sync.wait_ge(s_invs[h], 16 * (qp + 1))
                        sync.dma_start(
                            out=invb[:, h, :],
                            in_=inv_scr.ap()[qp, h, :, :].to_broadcast([64, QB]),
                        ).then_inc(s_invb[h], 16)
                    # head B partition shift tmp -> ot rows 64..127
                    if qb >= 2 and pair == 0:
                        sync.wait_ge(s["s_dn"], 8 * (qb - 1))
                    sync.wait_ge(s["s_nrm"], 2 * qp + 2)
                    sync.dma_start(
                        out=ot[64:128, qb % 2, pair, :], in_=tmp_sb[:, qp % 2, :]
                    ).then_inc(s_ot[qp % 2], 16)
                # output DMAs
                for m8 in range(KD):
                    d = qb * KD + m8
                    sync.wait_ge(s["s_stg"], d + 1)
                    sync.dma_start(
                        out=outT[m8 * 128:(m8 + 1) * 128, qb * QB:(qb + 1) * QB],
                        in_=stage[:, qb % 2, m8, :],
                    ).then_inc(s_out[qb % 2], 16)

        # ---------------- PE: matmuls + transposes ----------------
        @block.tensor
        def _(tensor):
            tensor.wait_ge(s["s_in"], 144)
            # phase 1
            for n in range(NQB):
                nsl = slice(n * QB, (n + 1) * QB)
                if n >= 1:
                    tensor.wait_ge(s["s_cp"], 6 * n)  # psum accumulators freed
                for k in range(KD):
                    i = n * KD + k
                    tensor.wait_ge(s_x[i % NX], 16 * (i // NX + 1))
                    last = None
                    for w_sb, _, w in projs:
                        for ct in range(CT):
                            last = nc.tensor.matmul(
                                psum[:, w * 2 + ct, :],
                                w_sb[:, k, ct * 128:(ct + 1) * 128],
                                x_ring[:, i % NX, :],
                                start=(k == 0), stop=(k == KD - 1),
                            )
                    last.then_inc(s["s_xcons"], 1)
                # V transposes for this n: t = n*8 + ct*4 + j
                for ct in range(CT):
                    tensor.wait_ge(s["s_cp"], 6 * n + 5 + ct)  # vt_roll ready
                    for j in range(QB // 128):
                        t = n * (2 * (QB // 128)) + ct * (QB // 128) + j
                        if t >= 2:
                            tensor.wait_ge(s["s_trcp"], t - 1)
                        nc.tensor.transpose(
                            psum[:, 6 + t % 2, 0:128].bitcast(F32R),
                            vt_roll[:, (2 * n + ct) % 2, j * 128:(j + 1) * 128],
                            ident,
                        ).then_inc(s["s_tr"], 1)

            # phase 1 psum fully consumed before attention reuses the banks
            tensor.wait_ge(s["s_cp"], 6 * NQB)
            tensor.wait_ge(s["s_trcp"], 2 * (QB // 128) * NQB)

            # attention — software-pipelined: QK(e+1) issues before PV(e),
            # so PE never stalls on ScalarE's exp; PV accumulators are
            # spilled to SBUF by DVE (s_acs) so normalization (reciprocal +
            # DRAM-bounce broadcast) runs entirely off PE's critical path.
            def emit_qk(e):
                qp, m = divmod(e, ST_N)
                qb, pair = divmod(qp, NPAIR)
                b = e % 2
                msl = slice(m * 128, (m + 1) * 128)
                qsl = slice(qb * QB, (qb + 1) * QB)
                if e >= 2:
                    tensor.wait_ge(s["s_exp"], e - 1)
                nc.tensor.matmul(
                    psum[:, b * 2 + 0, :],
                    kt_sb[0:64, pair, msl],
                    qt_sb[0:64, pair, qsl],
                    start=True, stop=True, tile_position=(0, 0),
                )
                nc.tensor.matmul(
                    psum[:, b * 2 + 1, :],
                    kt_sb[64:128, pair, msl],
                    qt_sb[64:128, pair, qsl],
                    start=True, stop=True, tile_position=(64, 0),
                ).then_inc(s["s_st"], 1)

            def emit_pv(e):
                qp, m = divmod(e, ST_N)
                pair = qp % NPAIR
                tensor.wait_ge(s["s_exp"], e + 1)
                if m == 0 and qp >= 1:
                    tensor.wait_ge(s["s_acs"], 2 * qp)  # acc spilled to SBUF
                nc.tensor.matmul(
                    psum[0:65, 4, :],
                    vaug[:, m, 2 * pair, :],
                    p_ring[:, e % NP, 0, :],
                    start=(m == 0), stop=(m == ST_N - 1),
                )
                nc.tensor.matmul(
                    psum[0:65, 5, :],
                    vaug[:, m, 2 * pair + 1, :],
                    p_ring[:, e % NP, 1, :],
                    start=(m == 0), stop=(m == ST_N - 1),
                ).then_inc(s["s_pcons"], 1)

            def emit_dense(qb):
                tensor.wait_ge(s["s_nrm"], 4 * qb + 4)
                tensor.wait_ge(s_ot[0], 16 * (qb + 1))
                tensor.wait_ge(s_ot[1], 16 * (qb + 1))
                for m8 in range(KD):
                    d = qb * KD + m8
                    if d >= 2:
                        tensor.wait_ge(s["s_stg"], d - 1)
                    nc.tensor.matmul(
                        psum[:, 6 + d % 2, :],
                        dw_sb[:, 0, m8 * 128:(m8 + 1) * 128],
                        ot[:, qb % 2, 0, :],
                        start=True, stop=False,
                    )
                    nc.tensor.matmul(
                        psum[:, 6 + d % 2, :],
                        dw_sb[:, 1, m8 * 128:(m8 + 1) * 128],
                        ot[:, qb % 2, 1, :],
                        start=False, stop=True,
                    ).then_inc(s["s_dn"], 1)

            # dense(qb) is emitted one full pair after its inputs complete,
            # giving the normalization round-trip time to land in ot
            dense_at = {(2 * qb + 2) * ST_N + 8: qb for qb in range(NQB - 1)}
            for e in range(NE + 1):
                if e < NE:
                    emit_qk(e)
                if e >= 1:
                    emit_pv(e - 1)
                if e in dense_at:
                    emit_dense(dense_at[e])
            emit_dense(NQB - 1)

        # ---------------- ACT: exp ----------------
        @block.scalar
        def _(scalar):
            for e in range(NE):
                b = e % 2
                scalar.wait_ge(s["s_st"], e + 1)
                if e >= NP:
                    scalar.wait_ge(s["s_pcons"], e - (NP - 1))
                nc.scalar.activation(
                    out=p_ring[:, e % NP, :, :],
                    in_=psum[:, b * 2:b * 2 + 2, :],
                    func=AFT.Exp, scale=SCALE,
                ).then_inc(s["s_exp"], 1)

        # ---------------- DVE: bias adds, copies, normalize, stage ----------
        @block.vector
        def _(vector):
            for st_i in range(ST_N):
                for hh in range(TP):
                    nc.vector.memset(
                        vaug[:, st_i, hh, DEPTH:DEPTH + 1].bitcast(F32), 1.0
                    )
            vector.wait_ge(s["s_in"], 144)
            # phase 1
            for n in range(NQB):
                nsl = slice(n * QB, (n + 1) * QB)
                vector.wait_ge(s["s_xcons"], KD * (n + 1))
                for w_sb, b_sb, w in projs[:2]:
                    dst = qt_sb if w == 0 else kt_sb
                    for ct in range(CT):
                        nc.vector.tensor_scalar_add(
                            out=dst[:, ct, nsl],
                            in0=psum[:, w * 2 + ct, :],
                            scalar1=b_sb[:, ct, :],
                        ).then_inc(s["s_cp"], 1)
                for ct in range(CT):
                    # vt_roll WAR: transposes of the slot two groups back
                    g = 2 * n + ct
                    if g >= 2:
                        prev = g - 2  # = 2n'+ct'
                        tprev = (prev // 2) * (2 * (QB // 128)) \
                            + (prev % 2) * (QB // 128) + (QB // 128 - 1)
                        vector.wait_ge(s["s_tr"], tprev + 1)
                    nc.vector.tensor_scalar_add(
                        out=vt_roll[:, g % 2, :],
                        in0=psum[:, 4 + ct, :],
                        scalar1=bv_sb[:, ct, :],
                    ).then_inc(s["s_cp"], 1)
                # V_aug assembly from transposed tiles
                for ct in range(CT):
                    for j in range(QB // 128):
                        t = n * (2 * (QB // 128)) + ct * (QB // 128) + j
                        st_i = n * (QB // 128) + j
                        vector.wait_ge(s["s_tr"], t + 1)
                        nc.vector.tensor_copy(
                            out=vaug[:, st_i, 2 * ct, 0:DEPTH],
                            in_=psum[:, 6 + t % 2, 0:DEPTH],
                        )
                        nc.vector.tensor_copy(
                            out=vaug[:, st_i, 2 * ct + 1, 0:DEPTH],
                            in_=psum[:, 6 + t % 2, DEPTH:128],
                        ).then_inc(s["s_trcp"], 1)

            # attention: normalization + dense staging
            for qb in range(NQB):
                for pair in range(NPAIR):
                    qp = qb * NPAIR + pair
                    vector.wait_ge(s["s_pcons"], ST_N * (qp + 1))
                    for h in range(2):
                        nc.vector.tensor_copy(
                            out=ounorm[0:65, qp % 2, h, :],
                            in_=psum[0:65, 4 + h, :],
                        ).then_inc(s["s_acs"], 1)
                    # self-wait: DVE's deep pipe may start the next read
                    # before the spill writes retire; sem incs fire at retire
                    vector.wait_ge(s["s_acs"], 2 * (qp + 1))
                    for h in range(2):
                        if qp >= 1:
                            # inv_sb WAR: scratch DMA of previous qp done
                            vector.wait_ge(s_invs[h], 16 * qp)
                        nc.vector.reciprocal(
                            out=inv_sb[64:65, h, :],
                            in_=ounorm[64:65, qp % 2, h, :],
                        ).then_inc(s["s_inv"], 1)
                    for h in range(2):
                        if pair == 0 and h == 0 and qb >= 2:
                            vector.wait_ge(s["s_dn"], 8 * (qb - 1))  # ot WAR
                        vector.wait_ge(s_invb[h], 16 * (qp + 1))
                        if h == 0:
                            nc.vector.tensor_mul(
                                out=ot[0:64, qb % 2, pair, :],
                                in0=ounorm[0:64, qp % 2, 0, :], in1=invb[:, 0, :],
                            ).then_inc(s["s_nrm"], 1)
                        else:
                            if qp >= 2:
                                vector.wait_ge(s_ot[qp % 2], 16 * (qp // 2))
                            nc.vector.tensor_mul(
                                out=tmp_sb[:, qp % 2, :],
                                in0=ounorm[0:64, qp % 2, 1, :], in1=invb[:, 1, :],
                            ).then_inc(s["s_nrm"], 1)
                for m8 in range(KD):
                    d = qb * KD + m8
                    vector.wait_ge(s["s_dn"], d + 1)
                    if qb >= 2 and m8 == 0:
                        # stage slot WAR: all of q block qb-2's output DMAs
                        vector.wait_ge(s_out[qb % 2], 16 * KD * (qb // 2))
                    nc.vector.tensor_scalar_add(
                        out=stage[:, qb % 2, m8, :],
                        in0=psum[:, 6 + d % 2, :],
                        scalar1=db_sb[:, m8, :],
                    ).then_inc(s["s_stg"], 1)

    nc.finalize()
    return nc


_NC_CACHE = []


def get_nc():
    if not _NC_CACHE:
        _NC_CACHE.append(build_nc())
    return _NC_CACHE[0]


def make_in_maps(x, wq_w, wq_b, wk_w, wk_b, wv_w, wv_b, dense_w, dense_b):
    in_maps = []
    for core in range(8):
        b, g = divmod(core, TP)
        blk = slice(g * C, (g + 1) * C)
        db_g = dense_b if g == 0 else np.zeros_like(dense_b)
        in_maps.append({
            "xT": np.ascontiguousarray(x[b].T),
            "wqT": np.ascontiguousarray(wq_w[blk, :].T),
            "wkT": np.ascontiguousarray(wk_w[blk, :].T),
            "wvT": np.ascontiguousarray(wv_w[blk, :].T),
            "bq": np.ascontiguousarray(wq_b[blk].reshape(C, 1)),
            "bk": np.ascontiguousarray(wk_b[blk].reshape(C, 1)),
            "bv": np.ascontiguousarray(wv_b[blk].reshape(C, 1)),
            "dwT": np.ascontiguousarray(dense_w[:, blk].T),
            "db": np.ascontiguousarray(db_g.reshape(D, 1)),
            "identity": np.eye(128, dtype=np.float32),
        })
    return in_maps


def gather_out(results):
    out = np.zeros((B, S, D), dtype=np.float32)
    for core in range(8):
        b = core // TP
        out[b] += results[core]["outT"].T
    return out


def kernel(x, wq_w, wq_b, wk_w, wk_b, wv_w, wv_b, dense_w, dense_b, **run_kwargs):
    args = [np.asarray(a, dtype=np.float32) for a in (
        x, wq_w, wq_b, wk_w, wk_b, wv_w, wv_b, dense_w, dense_b)]
    nc = get_nc()
    in_maps = make_in_maps(*args)
    res = run_bass_kernel_spmd(nc, in_maps, list(range(8)), **run_kwargs)
    out = gather_out(res.results)
    kernel.last_results = res
    return out


# revision 24
# speedup vs baseline: 1.8728x; 1.0668x over previous
"""Multi-head self-attention on 8 Trainium2 NeuronCores (raw Bass).

Problem: B=2, S=2048, D=1024, H=16 heads of depth 64 (fp32).
    q/k/v = x @ W.T + b ; per-head softmax(q k^T / 8) v ; dense out proj.

Sharding: DP-2 on batch x TP-4 on heads. Core (b, g) handles batch b and the
4 heads g*4..g*4+3 (a 256-wide column block of q/k/v). The dense layer is
row-split over the same block, so each core emits a partial [S, D] output;
the host sums the 4 partials per batch (dense bias rides on the g==0 cores).

Raw-bass implementation (the toolchain accepts at most ONE semaphore wait
per engine instruction, so Tile's multi-wait sync is unusable; explicit
wait_ge instructions + then_inc updates are used throughout):

  phase 1: stream x^T in [128, 512] slices; QT/KT/VT = W_T.T @ x^T in [c, s]
           layout (f32r matmuls, 6 psum accumulators); VT slices are
           PE-transposed into V_aug [s, c] with a ones column appended.
  attention (8 iters = head pair x 512-wide q block; 16 k tiles each):
           scores^T via row-packed pair matmuls (contraction=64, two heads
           share the 128-row PE array), exp on ScalarE (scale=1/8 folded in),
           PV accumulation with lhsT = V_aug; its ones column leaves the
           softmax denominators in psum row 64; normalize via reciprocal +
           a DRAM-bounce partition broadcast.
  dense:   per q block, partial^T = dwT.T @ OT (+ db) -> DMA out.
"""

import numpy as np
import sys

if "/opt/trn_rl_repo" not in sys.path:
    sys.path.insert(0, "/opt/trn_rl_repo")

import concourse.bass as bass
from concourse import mybir
from concourse.bass_utils import run_bass_kernel_spmd

F32 = mybir.dt.float32
F32R = mybir.dt.float32r
AFT = mybir.ActivationFunctionType

B, S, D = 2, 2048, 1024
H, DEPTH = 16, 64
TP = 4                     # head-parallel groups
C = D // TP                # 256 cols per core (4 heads)
CT = C // 128              # 2 partition tiles of the head block
KD = D // 128              # 8 contraction tiles for projections
ST_N = S // 128            # 16 s/k tiles
QB = 512                   # q block width
NQB = S // QB              # 4 q blocks
NPAIR = 2                  # head pairs per core
NITER = NQB * NPAIR        # 8 attention iterations (qp)
NE = NITER * ST_N          # 128 score-tile steps (e)
NX = 4                     # x slice ring depth
NP = 4                     # p tile ring depth
SCALE = float(1.0 / np.sqrt(DEPTH))


def build_nc():
    nc = bass.Bass()

    xT = nc.dram_tensor("xT", [D, S], F32R, kind="ExternalInput")
    wqT = nc.dram_tensor("wqT", [D, C], F32R, kind="ExternalInput")
    wkT = nc.dram_tensor("wkT", [D, C], F32R, kind="ExternalInput")
    wvT = nc.dram_tensor("wvT", [D, C], F32R, kind="ExternalInput")
    bq = nc.dram_tensor("bq", [C, 1], F32, kind="ExternalInput")
    bk = nc.dram_tensor("bk", [C, 1], F32, kind="ExternalInput")
    bv = nc.dram_tensor("bv", [C, 1], F32, kind="ExternalInput")
    dwT = nc.dram_tensor("dwT", [C, D], F32R, kind="ExternalInput")
    db = nc.dram_tensor("db", [D, 1], F32, kind="ExternalInput")
    identity = nc.dram_tensor("identity", [128, 128], F32R, kind="ExternalInput")
    outT = nc.dram_tensor("outT", [D, S], F32, kind="ExternalOutput")
    inv_scr = nc.dram_tensor("inv_scr", [NITER, 2, 1, QB], F32)

    # ---- SBUF ----
    wq_sb = nc.alloc_sbuf_tensor("wq_sb", [128, KD, C], F32R).ap()
    wk_sb = nc.alloc_sbuf_tensor("wk_sb", [128, KD, C], F32R).ap()
    wv_sb = nc.alloc_sbuf_tensor("wv_sb", [128, KD, C], F32R).ap()
    bq_sb = nc.alloc_sbuf_tensor("bq_sb", [128, CT, 1], F32).ap()
    bk_sb = nc.alloc_sbuf_tensor("bk_sb", [128, CT, 1], F32).ap()
    bv_sb = nc.alloc_sbuf_tensor("bv_sb", [128, CT, 1], F32).ap()
    dw_sb = nc.alloc_sbuf_tensor("dw_sb", [128, CT, D], F32R).ap()
    db_sb = nc.alloc_sbuf_tensor("db_sb", [128, KD, 1], F32).ap()
    ident = nc.alloc_sbuf_tensor("ident", [128, 128], F32R).ap()
    x_ring = nc.alloc_sbuf_tensor("x_ring", [128, NX, QB], F32R).ap()
    qt_sb = nc.alloc_sbuf_tensor("qt_sb", [128, CT, S], F32R).ap()
    kt_sb = nc.alloc_sbuf_tensor("kt_sb", [128, CT, S], F32R).ap()
    vt_roll = nc.alloc_sbuf_tensor("vt_roll", [128, 2, QB], F32R).ap()
    vaug = nc.alloc_sbuf_tensor("vaug", [128, ST_N, TP, DEPTH + 1], F32R).ap()
    p_ring = nc.alloc_sbuf_tensor("p_ring", [128, NP, 2, QB], F32R).ap()
    inv_sb = nc.alloc_sbuf_tensor("inv_sb", [128, 2, QB], F32).ap()
    invb = nc.alloc_sbuf_tensor("invb", [64, 2, QB], F32).ap()
    tmp_sb = nc.alloc_sbuf_tensor("tmp_sb", [64, 2, QB], F32R).ap()
    ot = nc.alloc_sbuf_tensor("ot", [128, 2, CT, QB], F32R).ap()
    ounorm = nc.alloc_sbuf_tensor("ounorm", [128, 2, 2, QB], F32).ap()
    stage = nc.alloc_sbuf_tensor("stage", [128, 2, KD, QB], F32).ap()

    # ---- PSUM: one [128, 8, 512] tensor, banks managed manually ----
    # phase 1: banks 0-5 = projection accumulators (w*2+ct), 6-7 = transposes
    # attention: banks 0-3 = score tiles (slot b*2+head), 4-5 = PV accum A/B,
    #            6-7 = dense ring
    psum = nc.alloc_psum_tensor("ps", [128, 8, QB], F32).ap()

    # ---- semaphores ----
    # DMA completions across HW queues are out-of-order, so every DMA
    # stream with >1 outstanding transfer gets per-slot semaphores: each
    # wait value then corresponds to a deterministic set of completions.
    s = {n: nc.alloc_semaphore(n) for n in (
        "s_wq", "s_wk", "s_wv", "s_misc", "s_xcons", "s_cp", "s_tr", "s_trcp",
        "s_st", "s_exp", "s_pcons", "s_acs", "s_nrm", "s_inv", "s_dn",
        "s_stg")}
    s_x = [nc.alloc_semaphore(f"s_x{j}") for j in range(NX)]
    s_invs = [nc.alloc_semaphore(f"s_invs{h}") for h in range(2)]
    s_invb = [nc.alloc_semaphore(f"s_invb{h}") for h in range(2)]
    s_ot = [nc.alloc_semaphore(f"s_ot{p}") for p in range(2)]
    s_out = [nc.alloc_semaphore(f"s_out{p}") for p in range(2)]

    projs = ((wq_sb, bq_sb, 0), (wk_sb, bk_sb, 1), (wv_sb, bv_sb, 2))

    with nc.Block() as block:

        # ---------------- SP: all HWDGE DMA traffic ----------------
        @block.sync
        def _(sync):
            # inputs: wq + first x slices first so PE starts ASAP
            def emit_x(i):
                n, k = divmod(i, KD)
                if i >= NX:
                    sync.wait_ge(s["s_xcons"], i - (NX - 1))
                sync.dma_start(
                    out=x_ring[:, i % NX, :],
                    in_=xT[k * 128:(k + 1) * 128, n * QB:(n + 1) * QB],
                ).then_inc(s_x[i % NX], 16)

            sync.dma_start(
                out=wq_sb, in_=wqT.ap().rearrange("(k p) c -> p k c", p=128)
            ).then_inc(s["s_wq"], 16)
            for i in range(NX):
                emit_x(i)
            sync.dma_start(
                out=wk_sb, in_=wkT.ap().rearrange("(k p) c -> p k c", p=128)
            ).then_inc(s["s_wk"], 16)
            sync.dma_start(
                out=wv_sb, in_=wvT.ap().rearrange("(k p) c -> p k c", p=128)
            ).then_inc(s["s_wv"], 16)
            with nc.allow_non_contiguous_dma(reason="tiny bias vectors"):
                sync.dma_start(
                    out=bq_sb, in_=bq.ap().rearrange("(ct p) o -> p ct o", p=128)
                ).then_inc(s["s_misc"], 16)
                sync.dma_start(
                    out=bk_sb, in_=bk.ap().rearrange("(ct p) o -> p ct o", p=128)
                ).then_inc(s["s_misc"], 16)
                sync.dma_start(
                    out=bv_sb, in_=bv.ap().rearrange("(ct p) o -> p ct o", p=128)
                ).then_inc(s["s_misc"], 16)
            sync.dma_start(
                out=dw_sb, in_=dwT.ap().rearrange("(ct p) e -> p ct e", p=128)
            ).then_inc(s["s_misc"], 16)
            with nc.allow_non_contiguous_dma(reason="tiny bias vector"):
                sync.dma_start(
                    out=db_sb, in_=db.ap().rearrange("(m p) o -> p m o", p=128)
                ).then_inc(s["s_misc"], 16)
            sync.dma_start(out=ident, in_=identity.ap()).then_inc(s["s_misc"], 16)
            for i in range(NX, NQB * KD):
                emit_x(i)

            # attention-side DMA chains
            for qb in range(NQB):
                for pair in range(NPAIR):
                    qp = qb * NPAIR + pair
                    for h in range(2):
                        sync.wait_ge(s["s_inv"], 2 * qp + h + 1)
                        sync.dma_start(
                            out=inv_scr.ap()[qp, h, :, :],
                            in_=inv_sb[64:65, h, :],
                        ).then_inc(s_invs[h], 16)
                    for h in range(2):
                        if qp >= 1:
                            # WAR: previous mul must have read invb[h]
                            sync.wait_ge(s["s_nrm"], 2 * (qp - 1) + h + 1)
                        sync.wait_ge(s_invs[h], 16 * (qp + 1))
                        sync.dma_start(
                            out=invb[:, h, :],
                            in_=inv_scr.ap()[qp, h, :, :].to_broadcast([64, QB]),
                        ).then_inc(s_invb[h], 16)
                    # head B partition shift tmp -> ot rows 64..127
                    if qb >= 2 and pair == 0:
                        sync.wait_ge(s["s_dn"], 8 * (qb - 1))
                    sync.wait_ge(s["s_nrm"], 2 * qp + 2)
                    sync.dma_start(
                        out=ot[64:128, qb % 2, pair, :], in_=tmp_sb[:, qp % 2, :]
                    ).then_inc(s_ot[qp % 2], 16)
                # output DMAs
                for m8 in range(KD):
                    d = qb * KD + m8
                    sync.wait_ge(s["s_stg"], d + 1)
                    sync.dma_start(
                        out=outT[m8 * 128:(m8 + 1) * 128, qb * QB:(qb + 1) * QB],
                        in_=stage[:, qb % 2, m8, :],
                    ).then_inc(s_out[qb % 2], 16)

        # ---------------- PE: matmuls + transposes ----------------
        @block.tensor
        def _(tensor):
            # DVE bias-add order is v0,v1,q0,q1,k0,k1 (s_cp 6n+1..6n+6); PE
            # touches freed banks in that order so it never over-waits
            mm_order = ((2, 0), (2, 1), (0, 0), (0, 1), (1, 0), (1, 1))
            bank_free = {(2, 0): 1, (2, 1): 2, (0, 0): 3, (0, 1): 4,
                         (1, 0): 5, (1, 1): 6}
            w_sem = {0: "s_wq", 1: "s_wk", 2: "s_wv"}
            w_seen = set()
            for n in range(NQB):
                for k in range(KD):
                    i = n * KD + k
                    tensor.wait_ge(s_x[i % NX], 16 * (i // NX + 1))
                    last = None
                    for w, ct in mm_order:
                        if w not in w_seen:
                            w_seen.add(w)
                            tensor.wait_ge(s[w_sem[w]], 16)
                        if n >= 1 and k == 0:
                            tensor.wait_ge(s["s_cp"], 6 * (n - 1) + bank_free[w, ct])
                        last = nc.tensor.matmul(
                            psum[:, w * 2 + ct, :],
                            (projs[w][0])[:, k, ct * 128:(ct + 1) * 128],
                            x_ring[:, i % NX, :],
                            start=(k == 0), stop=(k == KD - 1),
                        )
                    last.then_inc(s["s_xcons"], 1)
                # V transposes for this n: t = n*8 + ct*4 + j
                if n == 0:
                    tensor.wait_ge(s["s_misc"], 96)  # identity loaded
                for ct in range(CT):
                    tensor.wait_ge(s["s_cp"], 6 * n + 1 + ct)  # vt_roll ready
                    for j in range(QB // 128):
                        t = n * (2 * (QB // 128)) + ct * (QB // 128) + j
                        if t >= 2:
                            tensor.wait_ge(s["s_trcp"], t - 1)
                        nc.tensor.transpose(
                            psum[:, 6 + t % 2, 0:128].bitcast(F32R),
                            vt_roll[:, (2 * n + ct) % 2, j * 128:(j + 1) * 128],
                            ident,
                        ).then_inc(s["s_tr"], 1)

            # phase 1 psum fully consumed before attention reuses the banks
            tensor.wait_ge(s["s_cp"], 6 * NQB)
            tensor.wait_ge(s["s_trcp"], 2 * (QB // 128) * NQB)

            # attention — software-pipelined: QK(e+1) issues before PV(e),
            # so PE never stalls on ScalarE's exp; PV accumulators are
            # spilled to SBUF by DVE (s_acs) so normalization (reciprocal +
            # DRAM-bounce broadcast) runs entirely off PE's critical path.
            def emit_qk(e):
                qp, m = divmod(e, ST_N)
                qb, pair = divmod(qp, NPAIR)
                b = e % 2
                msl = slice(m * 128, (m + 1) * 128)
                qsl = slice(qb * QB, (qb + 1) * QB)
                if e >= 2:
                    tensor.wait_ge(s["s_exp"], e - 1)
                nc.tensor.matmul(
                    psum[:, b * 2 + 0, :],
                    kt_sb[0:64, pair, msl],
                    qt_sb[0:64, pair, qsl],
                    start=True, stop=True, tile_position=(0, 0),
                )
                nc.tensor.matmul(
                    psum[:, b * 2 + 1, :],
                    kt_sb[64:128, pair, msl],
                    qt_sb[64:128, pair, qsl],
                    start=True, stop=True, tile_position=(64, 0),
                ).then_inc(s["s_st"], 1)

            def emit_pv(e):
                qp, m = divmod(e, ST_N)
                pair = qp % NPAIR
                tensor.wait_ge(s["s_exp"], e + 1)
                if m == 0 and qp >= 1:
                    tensor.wait_ge(s["s_acs"], 2 * qp)  # acc spilled to SBUF
                nc.tensor.matmul(
                    psum[0:65, 4, :],
                    vaug[:, m, 2 * pair, :],
                    p_ring[:, e % NP, 0, :],
                    start=(m == 0), stop=(m == ST_N - 1),
                )
                nc.tensor.matmul(
                    psum[0:65, 5, :],
                    vaug[:, m, 2 * pair + 1, :],
                    p_ring[:, e % NP, 1, :],
                    start=(m == 0), stop=(m == ST_N - 1),
                ).then_inc(s["s_pcons"], 1)

            def emit_dense_step(qb, m8):
                if m8 == 0:
                    tensor.wait_ge(s["s_nrm"], 4 * qb + 4)
                    tensor.wait_ge(s_ot[0], 16 * (qb + 1))
                    tensor.wait_ge(s_ot[1], 16 * (qb + 1))
                d = qb * KD + m8
                if d >= 2:
                    tensor.wait_ge(s["s_stg"], d - 1)
                nc.tensor.matmul(
                    psum[:, 6 + d % 2, :],
                    dw_sb[:, 0, m8 * 128:(m8 + 1) * 128],
                    ot[:, qb % 2, 0, :],
                    start=True, stop=False,
                )
                nc.tensor.matmul(
                    psum[:, 6 + d % 2, :],
                    dw_sb[:, 1, m8 * 128:(m8 + 1) * 128],
                    ot[:, qb % 2, 1, :],
                    start=False, stop=True,
                ).then_inc(s["s_dn"], 1)

            # dense(qb) is emitted one full pair after its inputs complete
            # (the normalization round-trip has landed in ot by then) and is
            # spread one m8-step per e to plug PE's slack vs ScalarE
            dense_at = {}
            for qb in range(NQB - 1):
                for m8 in range(KD):
                    dense_at[(2 * qb + 2) * ST_N + 6 + m8] = (qb, m8)
            for e in range(NE + 1):
                if e < NE:
                    emit_qk(e)
                if e >= 1:
                    emit_pv(e - 1)
                if e in dense_at:
                    emit_dense_step(*dense_at[e])
            for m8 in range(KD):
                emit_dense_step(NQB - 1, m8)

        # ---------------- ACT: exp ----------------
        @block.scalar
        def _(scalar):
            for e in range(NE):
                b = e % 2
                scalar.wait_ge(s["s_st"], e + 1)
                if e >= NP:
                    scalar.wait_ge(s["s_pcons"], e - (NP - 1))
                nc.scalar.activation(
                    out=p_ring[:, e % NP, :, :],
                    in_=psum[:, b * 2:b * 2 + 2, :],
                    func=AFT.Exp, scale=SCALE,
                ).then_inc(s["s_exp"], 1)

        # ---------------- DVE: bias adds, copies, normalize, stage ----------
        @block.vector
        def _(vector):
            for st_i in range(ST_N):
                for hh in range(TP):
                    nc.vector.memset(
                        vaug[:, st_i, hh, DEPTH:DEPTH + 1].bitcast(F32), 1.0
                    )
            vector.wait_ge(s["s_misc"], 96)
            # phase 1 (v adds first: PE's transposes + next group wait on them)
            for n in range(NQB):
                nsl = slice(n * QB, (n + 1) * QB)
                vector.wait_ge(s["s_xcons"], KD * (n + 1))
                for ct in range(CT):
                    # vt_roll WAR: transposes of the slot two groups back
                    g = 2 * n + ct
                    if g >= 2:
                        prev = g - 2  # = 2n'+ct'
                        tprev = (prev // 2) * (2 * (QB // 128)) \
                            + (prev % 2) * (QB // 128) + (QB // 128 - 1)
                        vector.wait_ge(s["s_tr"], tprev + 1)
                    nc.vector.tensor_scalar_add(
                        out=vt_roll[:, g % 2, :],
                        in0=psum[:, 4 + ct, :],
                        scalar1=bv_sb[:, ct, :],
                    ).then_inc(s["s_cp"], 1)
                for w_sb, b_sb, w in projs[:2]:
                    dst = qt_sb if w == 0 else kt_sb
                    for ct in range(CT):
                        nc.vector.tensor_scalar_add(
                            out=dst[:, ct, nsl],
                            in0=psum[:, w * 2 + ct, :],
                            scalar1=b_sb[:, ct, :],
                        ).then_inc(s["s_cp"], 1)
                # V_aug assembly from transposed tiles
                for ct in range(CT):
                    for j in range(QB // 128):
                        t = n * (2 * (QB // 128)) + ct * (QB // 128) + j
                        st_i = n * (QB // 128) + j
                        vector.wait_ge(s["s_tr"], t + 1)
                        nc.vector.tensor_copy(
                            out=vaug[:, st_i, 2 * ct, 0:DEPTH],
                            in_=psum[:, 6 + t % 2, 0:DEPTH],
                        )
                        nc.vector.tensor_copy(
                            out=vaug[:, st_i, 2 * ct + 1, 0:DEPTH],
                            in_=psum[:, 6 + t % 2, DEPTH:128],
                        ).then_inc(s["s_trcp"], 1)

            # attention: normalization + dense staging
            for qb in range(NQB):
                for pair in range(NPAIR):
                    qp = qb * NPAIR + pair
                    vector.wait_ge(s["s_pcons"], ST_N * (qp + 1))
                    for h in range(2):
                        nc.vector.tensor_copy(
                            out=ounorm[0:65, qp % 2, h, :],
                            in_=psum[0:65, 4 + h, :],
                        ).then_inc(s["s_acs"], 1)
                    # self-wait: DVE's deep pipe may start the next read
                    # before the spill writes retire; sem incs fire at retire
                    vector.wait_ge(s["s_acs"], 2 * (qp + 1))
                    for h in range(2):
                        if qp >= 1:
                            # inv_sb WAR: scratch DMA of previous qp done
                            vector.wait_ge(s_invs[h], 16 * qp)
                        nc.vector.reciprocal(
                            out=inv_sb[64:65, h, :],
                            in_=ounorm[64:65, qp % 2, h, :],
                        ).then_inc(s["s_inv"], 1)
                    for h in range(2):
                        if pair == 0 and h == 0 and qb >= 2:
                            vector.wait_ge(s["s_dn"], 8 * (qb - 1))  # ot WAR
                        vector.wait_ge(s_invb[h], 16 * (qp + 1))
                        if h == 0:
                            nc.vector.tensor_mul(
                                out=ot[0:64, qb % 2, pair, :],
                                in0=ounorm[0:64, qp % 2, 0, :], in1=invb[:, 0, :],
                            ).then_inc(s["s_nrm"], 1)
                        else:
                            if qp >= 2:
                                vector.wait_ge(s_ot[qp % 2], 16 * (qp // 2))
                            nc.vector.tensor_mul(
                                out=tmp_sb[:, qp % 2, :],
                                in0=ounorm[0:64, qp % 2, 1, :], in1=invb[:, 1, :],
                            ).then_inc(s["s_nrm"], 1)
                for m8 in range(KD):
                    d = qb * KD + m8
                    vector.wait_ge(s["s_dn"], d + 1)
                    if qb >= 2 and m8 == 0:
                        # stage slot WAR: all of q block qb-2's output DMAs
                        vector.wait_ge(s_out[qb % 2], 16 * KD * (qb // 2))
                    nc.vector.tensor_scalar_add(
                        out=stage[:, qb % 2, m8, :],
                        in0=psum[:, 6 + d % 2, :],
                        scalar1=db_sb[:, m8, :],
                    ).then_inc(s["s_stg"], 1)

    nc.finalize()
    return nc


_NC_CACHE = []


def get_nc():
    if not _NC_CACHE:
        _NC_CACHE.append(build_nc())
    return _NC_CACHE[0]


def make_in_maps(x, wq_w, wq_b, wk_w, wk_b, wv_w, wv_b, dense_w, dense_b):
    in_maps = []
    for core in range(8):
        b, g = divmod(core, TP)
        blk = slice(g * C, (g + 1) * C)
        db_g = dense_b if g == 0 else np.zeros_like(dense_b)
        in_maps.append({
            "xT": np.ascontiguousarray(x[b].T),
            "wqT": np.ascontiguousarray(wq_w[blk, :].T),
            "wkT": np.ascontiguousarray(wk_w[blk, :].T),
            "wvT": np.ascontiguousarray(wv_w[blk, :].T),
            "bq": np.ascontiguousarray(wq_b[blk].reshape(C, 1)),
            "bk": np.ascontiguousarray(wk_b[blk].reshape(C, 1)),
            "bv": np.ascontiguousarray(wv_b[blk].reshape(C, 1)),
            "dwT": np.ascontiguousarray(dense_w[:, blk].T),
            "db": np.ascontiguousarray(db_g.reshape(D, 1)),
            "identity": np.eye(128, dtype=np.float32),
        })
    return in_maps


def gather_out(results):
    out = np.zeros((B, S, D), dtype=np.float32)
    for core in range(8):
        b = core // TP
        out[b] += results[core]["outT"].T
    return out


def kernel(x, wq_w, wq_b, wk_w, wk_b, wv_w, wv_b, dense_w, dense_b, **run_kwargs):
    args = [np.asarray(a, dtype=np.float32) for a in (
        x, wq_w, wq_b, wk_w, wk_b, wv_w, wv_b, dense_w, dense_b)]
    nc = get_nc()
    in_maps = make_in_maps(*args)
    res = run_bass_kernel_spmd(nc, in_maps, list(range(8)), **run_kwargs)
    out = gather_out(res.results)
    kernel.last_results = res
    return out


# revision 28
# speedup vs baseline: 1.9311x; 1.0312x over previous
"""Multi-head self-attention on 8 Trainium2 NeuronCores (raw Bass).

Problem: B=2, S=2048, D=1024, H=16 heads of depth 64 (fp32).
    q/k/v = x @ W.T + b ; per-head softmax(q k^T / 8) v ; dense out proj.

Sharding: DP-2 on batch x TP-4 on heads. Core (b, g) handles batch b and the
4 heads g*4..g*4+3 (a 256-wide column block of q/k/v). The dense layer is
row-split over the same block, so each core emits a partial [S, D] output;
the host sums the 4 partials per batch (dense bias rides on the g==0 cores).

Raw-bass implementation (the toolchain accepts at most ONE semaphore wait
per engine instruction, so Tile's multi-wait sync is unusable; explicit
wait_ge instructions + then_inc updates are used throughout):

  phase 1: stream x^T in [128, 512] slices; QT/KT/VT = W_T.T @ x^T in [c, s]
           layout (f32r matmuls, 6 psum accumulators); VT slices are
           PE-transposed into V_aug [s, c] with a ones column appended.
  attention (8 iters = head pair x 512-wide q block; 16 k tiles each):
           scores^T via row-packed pair matmuls (contraction=64, two heads
           share the 128-row PE array), exp on ScalarE (scale=1/8 folded in),
           PV accumulation with lhsT = V_aug; its ones column leaves the
           softmax denominators in psum row 64; normalize via reciprocal +
           a DRAM-bounce partition broadcast.
  dense:   per q block, partial^T = dwT.T @ OT (+ db) -> DMA out.
"""

import numpy as np
import sys

if "/opt/trn_rl_repo" not in sys.path:
    sys.path.insert(0, "/opt/trn_rl_repo")

import concourse.bass as bass
from concourse import mybir
from concourse.bass_utils import run_bass_kernel_spmd

F32 = mybir.dt.float32
F32R = mybir.dt.float32r
AFT = mybir.ActivationFunctionType

B, S, D = 2, 2048, 1024
H, DEPTH = 16, 64
TP = 4                     # head-parallel groups
C = D // TP                # 256 cols per core (4 heads)
CT = C // 128              # 2 partition tiles of the head block
KD = D // 128              # 8 contraction tiles for projections
ST_N = S // 128            # 16 s/k tiles
QB = 512                   # q block width
NQB = S // QB              # 4 q blocks
NPAIR = 2                  # head pairs per core
NITER = NQB * NPAIR        # 8 attention iterations (qp)
NE = NITER * ST_N          # 128 score-tile steps (e)
NX = 4                     # x slice ring depth
NP = 4                     # p tile ring depth
SCALE = float(1.0 / np.sqrt(DEPTH))


def build_nc():
    nc = bass.Bass()

    xT = nc.dram_tensor("xT", [D, S], F32R, kind="ExternalInput")
    wqT = nc.dram_tensor("wqT", [D, C], F32R, kind="ExternalInput")
    wkT = nc.dram_tensor("wkT", [D, C], F32R, kind="ExternalInput")
    wvT = nc.dram_tensor("wvT", [D, C], F32R, kind="ExternalInput")
    bq = nc.dram_tensor("bq", [C, 1], F32, kind="ExternalInput")
    bk = nc.dram_tensor("bk", [C, 1], F32, kind="ExternalInput")
    bv = nc.dram_tensor("bv", [C, 1], F32, kind="ExternalInput")
    dwT = nc.dram_tensor("dwT", [C, D], F32R, kind="ExternalInput")
    db = nc.dram_tensor("db", [D, 1], F32, kind="ExternalInput")
    identity = nc.dram_tensor("identity", [128, 128], F32R, kind="ExternalInput")
    outT = nc.dram_tensor("outT", [D, S], F32, kind="ExternalOutput")
    inv_scr = nc.dram_tensor("inv_scr", [NITER, 2, 1, QB], F32)

    # ---- SBUF ----
    wq_sb = nc.alloc_sbuf_tensor("wq_sb", [128, KD, C], F32R).ap()
    wk_sb = nc.alloc_sbuf_tensor("wk_sb", [128, KD, C], F32R).ap()
    wv_sb = nc.alloc_sbuf_tensor("wv_sb", [128, KD, C], F32R).ap()
    bq_sb = nc.alloc_sbuf_tensor("bq_sb", [128, CT, 1], F32).ap()
    bk_sb = nc.alloc_sbuf_tensor("bk_sb", [128, CT, 1], F32).ap()
    bv_sb = nc.alloc_sbuf_tensor("bv_sb", [128, CT, 1], F32).ap()
    dw_sb = nc.alloc_sbuf_tensor("dw_sb", [128, CT, D], F32R).ap()
    db_sb = nc.alloc_sbuf_tensor("db_sb", [128, KD, 1], F32).ap()
    ident = nc.alloc_sbuf_tensor("ident", [128, 128], F32R).ap()
    x_ring = nc.alloc_sbuf_tensor("x_ring", [128, NX, QB], F32R).ap()
    qt_sb = nc.alloc_sbuf_tensor("qt_sb", [128, CT, S], F32R).ap()
    kt_sb = nc.alloc_sbuf_tensor("kt_sb", [128, CT, S], F32R).ap()
    vt_roll = nc.alloc_sbuf_tensor("vt_roll", [128, 2, QB], F32R).ap()
    vaug = nc.alloc_sbuf_tensor("vaug", [128, ST_N, TP, DEPTH + 1], F32R).ap()
    p_ring = nc.alloc_sbuf_tensor("p_ring", [128, NP, 2, QB], F32R).ap()
    inv_sb = nc.alloc_sbuf_tensor("inv_sb", [128, 2, QB], F32).ap()
    invb = nc.alloc_sbuf_tensor("invb", [64, 2, QB], F32).ap()
    tmp_sb = nc.alloc_sbuf_tensor("tmp_sb", [64, 2, QB], F32R).ap()
    ot = nc.alloc_sbuf_tensor("ot", [128, 2, CT, QB], F32R).ap()
    ounorm = nc.alloc_sbuf_tensor("ounorm", [128, 2, 2, QB], F32).ap()
    stage = nc.alloc_sbuf_tensor("stage", [128, 2, KD, QB], F32).ap()

    # ---- PSUM: one [128, 8, 512] tensor, banks managed manually ----
    # phase 1: banks 0-5 = projection accumulators (w*2+ct), 6-7 = transposes
    # attention: banks 0-3 = score tiles (slot b*2+head), 4-5 = PV accum A/B,
    #            6-7 = dense ring
    psum = nc.alloc_psum_tensor("ps", [128, 8, QB], F32).ap()

    # ---- semaphores ----
    # DMA completions across HW queues are out-of-order, so every DMA
    # stream with >1 outstanding transfer gets per-slot semaphores: each
    # wait value then corresponds to a deterministic set of completions.
    s = {n: nc.alloc_semaphore(n) for n in (
        "s_wq", "s_wk", "s_wv", "s_misc", "s_xcons", "s_cpv", "s_cpqk", "s_tr",
        "s_trcp",
        "s_st", "s_exp", "s_pcons", "s_acs", "s_nrm", "s_inv", "s_dn",
        "s_stg")}
    s_x = [nc.alloc_semaphore(f"s_x{j}") for j in range(NX)]
    s_invs = [nc.alloc_semaphore(f"s_invs{h}") for h in range(2)]
    s_invb = [nc.alloc_semaphore(f"s_invb{h}") for h in range(2)]
    s_ot = [nc.alloc_semaphore(f"s_ot{p}") for p in range(2)]
    s_out = [nc.alloc_semaphore(f"s_out{p}") for p in range(2)]

    projs = ((wq_sb, bq_sb, 0), (wk_sb, bk_sb, 1), (wv_sb, bv_sb, 2))

    with nc.Block() as block:

        # ---------------- SP: all HWDGE DMA traffic ----------------
        @block.sync
        def _(sync):
            # inputs: wq + first x slices first so PE starts ASAP
            def emit_x(i):
                n, k = divmod(i, KD)
                if i >= NX:
                    sync.wait_ge(s["s_xcons"], i - (NX - 1))
                sync.dma_start(
                    out=x_ring[:, i % NX, :],
                    in_=xT[k * 128:(k + 1) * 128, n * QB:(n + 1) * QB],
                ).then_inc(s_x[i % NX], 16)

            sync.dma_start(
                out=wq_sb, in_=wqT.ap().rearrange("(k p) c -> p k c", p=128)
            ).then_inc(s["s_wq"], 16)
            for i in range(NX):
                emit_x(i)
            sync.dma_start(
                out=wk_sb, in_=wkT.ap().rearrange("(k p) c -> p k c", p=128)
            ).then_inc(s["s_wk"], 16)
            sync.dma_start(
                out=wv_sb, in_=wvT.ap().rearrange("(k p) c -> p k c", p=128)
            ).then_inc(s["s_wv"], 16)
            with nc.allow_non_contiguous_dma(reason="tiny bias vectors"):
                sync.dma_start(
                    out=bq_sb, in_=bq.ap().rearrange("(ct p) o -> p ct o", p=128)
                ).then_inc(s["s_misc"], 16)
                sync.dma_start(
                    out=bk_sb, in_=bk.ap().rearrange("(ct p) o -> p ct o", p=128)
                ).then_inc(s["s_misc"], 16)
                sync.dma_start(
                    out=bv_sb, in_=bv.ap().rearrange("(ct p) o -> p ct o", p=128)
                ).then_inc(s["s_misc"], 16)
            sync.dma_start(
                out=dw_sb, in_=dwT.ap().rearrange("(ct p) e -> p ct e", p=128)
            ).then_inc(s["s_misc"], 16)
            with nc.allow_non_contiguous_dma(reason="tiny bias vector"):
                sync.dma_start(
                    out=db_sb, in_=db.ap().rearrange("(m p) o -> p m o", p=128)
                ).then_inc(s["s_misc"], 16)
            sync.dma_start(out=ident, in_=identity.ap()).then_inc(s["s_misc"], 16)
            for i in range(NX, NQB * KD):
                emit_x(i)

            # attention-side DMA chains
            for qb in range(NQB):
                for pair in range(NPAIR):
                    qp = qb * NPAIR + pair
                    for h in range(2):
                        sync.wait_ge(s["s_inv"], 2 * qp + h + 1)
                        sync.dma_start(
                            out=inv_scr.ap()[qp, h, :, :],
                            in_=inv_sb[64:65, h, :],
                        ).then_inc(s_invs[h], 16)
                    for h in range(2):
                        if qp >= 1:
                            # WAR: previous mul must have read invb[h]
                            sync.wait_ge(s["s_nrm"], 2 * (qp - 1) + h + 1)
                        sync.wait_ge(s_invs[h], 16 * (qp + 1))
                        sync.dma_start(
                            out=invb[:, h, :],
                            in_=inv_scr.ap()[qp, h, :, :].to_broadcast([64, QB]),
                        ).then_inc(s_invb[h], 16)
                    # head B partition shift tmp -> ot rows 64..127
                    if qb >= 2 and pair == 0:
                        sync.wait_ge(s["s_dn"], 8 * (qb - 1))
                    sync.wait_ge(s["s_nrm"], 2 * qp + 2)
                    sync.dma_start(
                        out=ot[64:128, qb % 2, pair, :], in_=tmp_sb[:, qp % 2, :]
                    ).then_inc(s_ot[qp % 2], 16)
                # output DMAs
                for m8 in range(KD):
                    d = qb * KD + m8
                    sync.wait_ge(s["s_stg"], d + 1)
                    sync.dma_start(
                        out=outT[m8 * 128:(m8 + 1) * 128, qb * QB:(qb + 1) * QB],
                        in_=stage[:, qb % 2, m8, :],
                    ).then_inc(s_out[qb % 2], 16)

        # ---------------- PE: matmuls + transposes ----------------
        @block.tensor
        def _(tensor):
            # v-adds (DVE, s_cpv) free banks 4-5; q/k adds (ScalarE,
            # s_cpqk) free banks 0-3; PE touches banks in free order
            mm_order = ((2, 0), (2, 1), (0, 0), (0, 1), (1, 0), (1, 1))
            bank_free = {(2, 0): ("s_cpv", 2, 1), (2, 1): ("s_cpv", 2, 2),
                         (0, 0): ("s_cpqk", 4, 1), (0, 1): ("s_cpqk", 4, 2),
                         (1, 0): ("s_cpqk", 4, 3), (1, 1): ("s_cpqk", 4, 4)}
            w_sem = {0: "s_wq", 1: "s_wk", 2: "s_wv"}
            w_seen = set()
            for n in range(NQB):
                for k in range(KD):
                    i = n * KD + k
                    tensor.wait_ge(s_x[i % NX], 16 * (i // NX + 1))
                    last = None
                    for w, ct in mm_order:
                        if w not in w_seen:
                            w_seen.add(w)
                            tensor.wait_ge(s[w_sem[w]], 16)
                        if n >= 1 and k == 0:
                            sem, per, off = bank_free[w, ct]
                            tensor.wait_ge(s[sem], per * (n - 1) + off)
                        last = nc.tensor.matmul(
                            psum[:, w * 2 + ct, :],
                            (projs[w][0])[:, k, ct * 128:(ct + 1) * 128],
                            x_ring[:, i % NX, :],
                            start=(k == 0), stop=(k == KD - 1),
                        )
                    last.then_inc(s["s_xcons"], 1)
                # V transposes for this n: t = n*8 + ct*4 + j
                if n == 0:
                    tensor.wait_ge(s["s_misc"], 96)  # identity loaded
                for ct in range(CT):
                    tensor.wait_ge(s["s_cpv"], 2 * n + 1 + ct)  # vt_roll ready
                    for j in range(QB // 128):
                        t = n * (2 * (QB // 128)) + ct * (QB // 128) + j
                        if t >= 2:
                            tensor.wait_ge(s["s_trcp"], t - 1)
                        nc.tensor.transpose(
                            psum[:, 6 + t % 2, 0:128].bitcast(F32R),
                            vt_roll[:, (2 * n + ct) % 2, j * 128:(j + 1) * 128],
                            ident,
                        ).then_inc(s["s_tr"], 1)

            # phase 1 psum fully consumed before attention reuses the banks
            tensor.wait_ge(s["s_cpqk"], 4 * NQB)
            tensor.wait_ge(s["s_cpv"], 2 * NQB)
            tensor.wait_ge(s["s_trcp"], 2 * (QB // 128) * NQB)

            # attention — software-pipelined: QK(e+1) issues before PV(e),
            # so PE never stalls on ScalarE's exp; PV accumulators are
            # spilled to SBUF by DVE (s_acs) so normalization (reciprocal +
            # DRAM-bounce broadcast) runs entirely off PE's critical path.
            def emit_qk(e):
                qp, m = divmod(e, ST_N)
                qb, pair = divmod(qp, NPAIR)
                b = e % 2
                msl = slice(m * 128, (m + 1) * 128)
                qsl = slice(qb * QB, (qb + 1) * QB)
                if e >= 2:
                    tensor.wait_ge(s["s_exp"], e - 1)
                nc.tensor.matmul(
                    psum[:, b * 2 + 0, :],
                    kt_sb[0:64, pair, msl],
                    qt_sb[0:64, pair, qsl],
                    start=True, stop=True, tile_position=(0, 0),
                )
                nc.tensor.matmul(
                    psum[:, b * 2 + 1, :],
                    kt_sb[64:128, pair, msl],
                    qt_sb[64:128, pair, qsl],
                    start=True, stop=True, tile_position=(64, 0),
                ).then_inc(s["s_st"], 1)

            def emit_pv(e):
                qp, m = divmod(e, ST_N)
                pair = qp % NPAIR
                tensor.wait_ge(s["s_exp"], e + 1)
                if m == 0 and qp >= 1:
                    tensor.wait_ge(s["s_acs"], 2 * qp)  # acc spilled to SBUF
                nc.tensor.matmul(
                    psum[0:65, 4, :],
                    vaug[:, m, 2 * pair, :],
                    p_ring[:, e % NP, 0, :],
                    start=(m == 0), stop=(m == ST_N - 1),
                )
                nc.tensor.matmul(
                    psum[0:65, 5, :],
                    vaug[:, m, 2 * pair + 1, :],
                    p_ring[:, e % NP, 1, :],
                    start=(m == 0), stop=(m == ST_N - 1),
                ).then_inc(s["s_pcons"], 1)

            def emit_dense_step(qb, m8):
                if m8 == 0:
                    tensor.wait_ge(s["s_nrm"], 4 * qb + 4)
                    tensor.wait_ge(s_ot[0], 16 * (qb + 1))
                    tensor.wait_ge(s_ot[1], 16 * (qb + 1))
                d = qb * KD + m8
                if d >= 2:
                    tensor.wait_ge(s["s_stg"], d - 1)
                nc.tensor.matmul(
                    psum[:, 6 + d % 2, :],
                    dw_sb[:, 0, m8 * 128:(m8 + 1) * 128],
                    ot[:, qb % 2, 0, :],
                    start=True, stop=False,
                )
                nc.tensor.matmul(
                    psum[:, 6 + d % 2, :],
                    dw_sb[:, 1, m8 * 128:(m8 + 1) * 128],
                    ot[:, qb % 2, 1, :],
                    start=False, stop=True,
                ).then_inc(s["s_dn"], 1)

            # dense(qb) is emitted one full pair after its inputs complete
            # (the normalization round-trip has landed in ot by then) and is
            # spread one m8-step per e to plug PE's slack vs ScalarE
            dense_at = {(2 * qb + 2) * ST_N + 10: qb for qb in range(NQB - 1)}
            for e in range(NE + 1):
                if e < NE:
                    emit_qk(e)
                if e >= 1:
                    emit_pv(e - 1)
                if e in dense_at:
                    for m8 in range(KD):
                        emit_dense_step(dense_at[e], m8)
            for m8 in range(KD):
                emit_dense_step(NQB - 1, m8)

        # ---------------- ACT: q/k bias adds (phase 1) + exp ----------------
        @block.scalar
        def _(scalar):
            scalar.wait_ge(s["s_misc"], 96)
            for n in range(NQB):
                nsl = slice(n * QB, (n + 1) * QB)
                scalar.wait_ge(s["s_xcons"], KD * (n + 1))
                for w_sb, b_sb, w in projs[:2]:
                    dst = qt_sb if w == 0 else kt_sb
                    for ct in range(CT):
                        nc.scalar.activation(
                            out=dst[:, ct, nsl],
                            in_=psum[:, w * 2 + ct, :],
                            func=AFT.Identity, bias=b_sb[:, ct, :], scale=1.0,
                        ).then_inc(s["s_cpqk"], 1)
            for e in range(NE):
                b = e % 2
                scalar.wait_ge(s["s_st"], e + 1)
                if e >= NP:
                    scalar.wait_ge(s["s_pcons"], e - (NP - 1))
                nc.scalar.activation(
                    out=p_ring[:, e % NP, :, :],
                    in_=psum[:, b * 2:b * 2 + 2, :],
                    func=AFT.Exp, scale=SCALE,
                ).then_inc(s["s_exp"], 1)

        # ---------------- DVE: bias adds, copies, normalize, stage ----------
        @block.vector
        def _(vector):
            for st_i in range(ST_N):
                for hh in range(TP):
                    nc.vector.memset(
                        vaug[:, st_i, hh, DEPTH:DEPTH + 1].bitcast(F32), 1.0
                    )
            vector.wait_ge(s["s_misc"], 96)
            # phase 1 (v adds first: PE's transposes + next group wait on them)
            for n in range(NQB):
                nsl = slice(n * QB, (n + 1) * QB)
                vector.wait_ge(s["s_xcons"], KD * (n + 1))
                for ct in range(CT):
                    # vt_roll WAR: transposes of the slot two groups back
                    g = 2 * n + ct
                    if g >= 2:
                        prev = g - 2  # = 2n'+ct'
                        tprev = (prev // 2) * (2 * (QB // 128)) \
                            + (prev % 2) * (QB // 128) + (QB // 128 - 1)
                        vector.wait_ge(s["s_tr"], tprev + 1)
                    nc.vector.tensor_scalar_add(
                        out=vt_roll[:, g % 2, :],
                        in0=psum[:, 4 + ct, :],
                        scalar1=bv_sb[:, ct, :],
                    ).then_inc(s["s_cpv"], 1)
                # V_aug assembly from transposed tiles
                for ct in range(CT):
                    for j in range(QB // 128):
                        t = n * (2 * (QB // 128)) + ct * (QB // 128) + j
                        st_i = n * (QB // 128) + j
                        vector.wait_ge(s["s_tr"], t + 1)
                        nc.vector.tensor_copy(
                            out=vaug[:, st_i, 2 * ct, 0:DEPTH],
                            in_=psum[:, 6 + t % 2, 0:DEPTH],
                        )
                        nc.vector.tensor_copy(
                            out=vaug[:, st_i, 2 * ct + 1, 0:DEPTH],
                            in_=psum[:, 6 + t % 2, DEPTH:128],
                        ).then_inc(s["s_trcp"], 1)

            # attention: normalization + dense staging
            for qb in range(NQB):
                for pair in range(NPAIR):
                    qp = qb * NPAIR + pair
                    vector.wait_ge(s["s_pcons"], ST_N * (qp + 1))
                    for h in range(2):
                        nc.vector.tensor_copy(
                            out=ounorm[0:65, qp % 2, h, :],
                            in_=psum[0:65, 4 + h, :],
                        ).then_inc(s["s_acs"], 1)
                    # self-wait: DVE's deep pipe may start the next read
                    # before the spill writes retire; sem incs fire at retire
                    vector.wait_ge(s["s_acs"], 2 * (qp + 1))
                    for h in range(2):
                        if qp >= 1:
                            # inv_sb WAR: scratch DMA of previous qp done
                            vector.wait_ge(s_invs[h], 16 * qp)
                        nc.vector.reciprocal(
                            out=inv_sb[64:65, h, :],
                            in_=ounorm[64:65, qp % 2, h, :],
                        ).then_inc(s["s_inv"], 1)
                    for h in range(2):
                        if pair == 0 and h == 0 and qb >= 2:
                            vector.wait_ge(s["s_dn"], 8 * (qb - 1))  # ot WAR
                        vector.wait_ge(s_invb[h], 16 * (qp + 1))
                        if h == 0:
                            nc.vector.tensor_mul(
                                out=ot[0:64, qb % 2, pair, :],
                                in0=ounorm[0:64, qp % 2, 0, :], in1=invb[:, 0, :],
                            ).then_inc(s["s_nrm"], 1)
                        else:
                            if qp >= 2:
                                vector.wait_ge(s_ot[qp % 2], 16 * (qp // 2))
                            nc.vector.tensor_mul(
                                out=tmp_sb[:, qp % 2, :],
                                in0=ounorm[0:64, qp % 2, 1, :], in1=invb[:, 1, :],
                            ).then_inc(s["s_nrm"], 1)
                for m8 in range(KD):
                    d = qb * KD + m8
                    vector.wait_ge(s["s_dn"], d + 1)
                    if qb >= 2 and m8 == 0:
                        # stage slot WAR: all of q block qb-2's output DMAs
                        vector.wait_ge(s_out[qb % 2], 16 * KD * (qb // 2))
                    nc.vector.tensor_scalar_add(
                        out=stage[:, qb % 2, m8, :],
                        in0=psum[:, 6 + d % 2, :],
                        scalar1=db_sb[:, m8, :],
                    ).then_inc(s["s_stg"], 1)

    nc.finalize()
    return nc


_NC_CACHE = []


def get_nc():
    if not _NC_CACHE:
        _NC_CACHE.append(build_nc())
    return _NC_CACHE[0]


def make_in_maps(x, wq_w, wq_b, wk_w, wk_b, wv_w, wv_b, dense_w, dense_b):
    in_maps = []
    for core in range(8):
        b, g = divmod(core, TP)
        blk = slice(g * C, (g + 1) * C)
        db_g = dense_b if g == 0 else np.zeros_like(dense_b)
        in_maps.append({
            "xT": np.ascontiguousarray(x[b].T),
            "wqT": np.ascontiguousarray(wq_w[blk, :].T),
            "wkT": np.ascontiguousarray(wk_w[blk, :].T),
            "wvT": np.ascontiguousarray(wv_w[blk, :].T),
            "bq": np.ascontiguousarray(wq_b[blk].reshape(C, 1)),
            "bk": np.ascontiguousarray(wk_b[blk].reshape(C, 1)),
            "bv": np.ascontiguousarray(wv_b[blk].reshape(C, 1)),
            "dwT": np.ascontiguousarray(dense_w[:, blk].T),
            "db": np.ascontiguousarray(db_g.reshape(D, 1)),
            "identity": np.eye(128, dtype=np.float32),
        })
    return in_maps


def gather_out(results):
    out = np.zeros((B, S, D), dtype=np.float32)
    for core in range(8):
        b = core // TP
        out[b] += results[core]["outT"].T
    return out


def kernel(x, wq_w, wq_b, wk_w, wk_b, wv_w, wv_b, dense_w, dense_b, **run_kwargs):
    args = [np.asarray(a, dtype=np.float32) for a in (
        x, wq_w, wq_b, wk_w, wk_b, wv_w, wv_b, dense_w, dense_b)]
    nc = get_nc()
    in_maps = make_in_maps(*args)
    res = run_bass_kernel_spmd(nc, in_maps, list(range(8)), **run_kwargs)
    out = gather_out(res.results)
    kernel.last_results = res
    return out
